# revision 27
# baseline (speedup 1.0000x reference)
"""Trainium2 Bass kernel for BiochemicalDynamics.

Reference computation (f32):
    Ax    = A @ x                                   # [N, DIM]
    s     = R * rowsum(x * Ax)                      # [N, 1]
    out   = F - B*x - s                             # [N, DIM]

Strategy: compute Y^T = (A_local @ x)^T directly on the TensorEngine by
streaming A (host-side pre-transposed, fp8) as the MOVING operand
against stationary x row-chunks:

    Y^T[d, m] = sum_kc matmul(lhsT = x[kc*128:(kc+1)*128, :],     # [K, M=64]
                              rhs  = A^T[kc*128:(kc+1)*128, m])   # [K, N]

accumulated over all 64 K-chunks into a PSUM region [64, 1024].
fp8 DoubleRow packs two K-chunks per instruction (K=256), giving the PE
enough column rate to hide entirely under the A DMA stream.

The per-row dot s_i = R * <x_i, Y_i> needs a PARTITION-axis reduction
of z = x^T .* Y^T, done with tiny ones-vector matmuls (lhsT =
z[:, stripe], rhs = ones[64,1]) that land s in natural [128, .] layout.
The affine epilogue (v = F - s; out = -B*x + v) runs on the otherwise
idle VectorEngine via tensor_scalar with a per-partition scalar AP.

A is streamed m-major in pieces of (512, 256, 128, 128) output rows:
each piece's Y^T finishes while the next piece is still streaming, so
its reduction, epilogue and output store all hide under the DMA stream
— only the last (single-stripe) piece's short chain sits in the tail.

DMA discipline (hard-won): ALL loads go on the single SP HWDGE ring in
an order where every DMAHW sem lane's previous user completes early
(lanes are assigned round-robin across ALL HWDGE DMAs of both rings,
and a lane's next user's ISSUE is gated on its previous user's
completion). Mid-stream stores go via GPSIMD/SWDGE, which uses a
separate sem-lane pool, so their late gating can't stall the stream.

A is quantized to fp8_e4m3 host-side: its rounding error is random-sign
and averages out over the 8192-term row reductions (measured ~1.7e-3
max rel err vs the 2e-2 gate) while halving HBM traffic vs fp16 —
this kernel is DMA-bound on A (8MB/core at ~358 GB/s).

Sharding: row-shard A (1024 rows/core); every core gets the full x.
No cross-core communication.
"""

import sys

import numpy as np

for _p in ("/opt/trn_rl_repo", "/root/.axon_site/_ro/trn_rl_repo"):
    if _p not in sys.path:
        sys.path.append(_p)

N = 8192
DIM = 64
NCORES = 8
ROWS = N // NCORES       # 1024 rows of A per core
P = 128
NSTRIPES = ROWS // P     # 8 row-stripes per core
KC = N // P              # 64 contraction chunks of 128
KP = KC // 2             # 32 DoubleRow chunk-pairs

F_CONST = 1.0
B_CONST = 0.1
R_CONST = 0.01

# (m-offset, m-width, kc-chunk schedule). Widths stay within PSUM banks
# (512-aligned boundaries). The first chunk of piece 0 is small so the
# first matmul starts early; piece 3 tapers so the PE and the final
# chain catch the stream quickly at the end.
PIECES = (
    (0, 256, (8, 24, 32)),
    (256, 256, (32, 32)),
    (512, 256, (32, 32)),
    (768, 256, (16, 16, 16, 8, 8)),
)
P_STRIPES = ((0, 1), (2, 3), (4, 5), (6, 7))

_CACHE = {}


def _build_nc():
    import concourse.mybir as mybir
    import concourse.tile as tile
    from concourse import bacc

    f32 = mybir.dt.float32
    bf16 = mybir.dt.bfloat16
    f8 = mybir.dt.float8e4

    nc = bacc.Bacc(
        trn_type="TRN2", target_bir_lowering=False, debug=False, num_devices=NCORES
    )

    # at{p}[pp, kc, j] = A[rows_c[off_p + j], kc*128 + pp]  (fp8 A^T pieces)
    at_dram = [
        nc.dram_tensor(f"at{i}", [P, KC, w], f8, kind="ExternalInput")
        for i, (_, w, _) in enumerate(PIECES)
    ]
    # xs[p, kc, d] = x[kc*128 + p, d]          (stationary chunks, fp8)
    xs = nc.dram_tensor("xs", [P, KC, DIM], f8, kind="ExternalInput")
    # xtd[d, m] = x[rows_c[m], d]              (bf16, for the rowwise dot)
    xtd = nc.dram_tensor("xtd", [DIM, ROWS], bf16, kind="ExternalInput")
    # xl[p, s*64+d] = x[rows_c[s*128+p], d] - F/B   (bf16; F pre-folded so
    # the epilogue is a single tensor_scalar: out = -B*xl - s)
    xl = nc.dram_tensor("xl", [P, NSTRIPES * DIM], bf16, kind="ExternalInput")
    out = nc.dram_tensor("out", [P, NSTRIPES * DIM], f32, kind="ExternalOutput")

    mult = mybir.AluOpType.mult
    subtract = mybir.AluOpType.subtract
    DR = mybir.MatmulPerfMode.DoubleRow

    with tile.TileContext(nc) as tc:
        with (
            tc.tile_pool(name="xpool", bufs=1) as xpool,
            tc.tile_pool(name="spool", bufs=1) as spool,
            tc.tile_pool(name="psum", bufs=1, space="PSUM") as psum_pool,
        ):
            xs_sb = xpool.tile([P, KC, DIM], f8)
            xtd_sb = xpool.tile([DIM, ROWS], bf16)
            xl_sb = xpool.tile([P, NSTRIPES * DIM], bf16)
            ones_sb = xpool.tile([DIM, 1], bf16)
            nc.any.memset(ones_sb[:], 1.0)
            at_sb = [
                xpool.tile([P, KC, w], f8, name=f"at_sb{i}", tag=f"at{i}")
                for i, (_, w, _) in enumerate(PIECES)
            ]

            # Single-ring load order: the stationary piece the first
            # matmuls need, the first A chunk, the remaining small loads,
            # then the A stream.
            nc.sync.dma_start(out=xs_sb[:, 0:4, :], in_=xs[:, 0:4, :])
            w0 = PIECES[0][2][0]
            nc.sync.dma_start(out=at_sb[0][:, 0:w0, :], in_=at_dram[0][:, 0:w0, :])
            nc.sync.dma_start(out=xs_sb[:, 4:, :], in_=xs[:, 4:, :])
            nc.sync.dma_start(out=xtd_sb[:], in_=xtd[:])
            nc.sync.dma_start(out=xl_sb[:], in_=xl[:])
            for i, (_, w, chunks) in enumerate(PIECES):
                o = w0 if i == 0 else 0
                for ck in chunks[1 if i == 0 else 0 :]:
                    nc.sync.dma_start(
                        out=at_sb[i][:, o : o + ck, :],
                        in_=at_dram[i][:, o : o + ck, :],
                    )
                    o += ck

            # Y^T accumulation: [64, 1024] f32 PSUM (2 banks).
            yt_ps = psum_pool.tile([DIM, ROWS], f32, tag="yt")
            z_sb = spool.tile([DIM, ROWS], bf16, tag="z")
            s_ps = psum_pool.tile([P, NSTRIPES], f32, tag="s")
            o_sb = spool.tile([P, NSTRIPES * DIM], f32, tag="o")

            def stripe_reduce(s):
                # PE: s[p, s] = sum_d z[d, s*128 + p] via a ones-matmul.
                nc.tensor.matmul(
                    s_ps[:, s : s + 1],
                    z_sb[:, s * P : (s + 1) * P],
                    ones_sb[:],
                    start=True,
                    stop=True,
                )

            def stripe_out(s):
                # DVE: out = (xl_adj * -B) - s, with F pre-folded into
                # xl_adj = x - F/B host-side and R folded into z.
                nc.vector.tensor_scalar(
                    o_sb[:, s * DIM : (s + 1) * DIM],
                    xl_sb[:, s * DIM : (s + 1) * DIM],
                    -B_CONST,
                    s_ps[:, s : s + 1],
                    op0=mult,
                    op1=subtract,
                )

            def finish(i):
                stripes = P_STRIPES[i]
                for s in stripes:
                    stripe_reduce(s)
                for s in stripes:
                    stripe_out(s)
                s0, s1 = stripes[0], stripes[-1] + 1
                ring = nc.sync if i == len(PIECES) - 1 else nc.gpsimd
                ring.dma_start(
                    out=out[:, s0 * DIM : s1 * DIM],
                    in_=o_sb[:, s0 * DIM : s1 * DIM],
                )

            last = len(PIECES) - 1
            for i, (off, w, _) in enumerate(PIECES):
                for c in range(KP):
                    nc.tensor.matmul(
                        yt_ps[:, off : off + w],
                        xs_sb[:, 2 * c : 2 * c + 2, :],
                        at_sb[i][:, 2 * c : 2 * c + 2, :],
                        start=(c == 0),
                        stop=(c == KP - 1),
                        perf_mode=DR,
                    )
                # z[d, m] = (xtd * R) * Y^T for this piece -> bf16
                nc.vector.scalar_tensor_tensor(
                    z_sb[:, off : off + w],
                    xtd_sb[:, off : off + w],
                    R_CONST,
                    yt_ps[:, off : off + w],
                    op0=mult,
                    op1=mult,
                )
                # Finish piece i-1 AFTER piece i's matmuls so the PE
                # never stalls waiting on the DVE mid-stream.
                if i > 0:
                    finish(i - 1)
            finish(last)

    nc.finalize()
    return nc


def _get_nc():
    if "nc" not in _CACHE:
        _CACHE["nc"] = _build_nc()
    return _CACHE["nc"]


def _make_in_maps(x, A):
    import ml_dtypes

    f8 = ml_dtypes.float8_e4m3
    bf16 = ml_dtypes.bfloat16
    x = np.ascontiguousarray(np.asarray(x, dtype=np.float32))
    A = np.asarray(A, dtype=np.float32)

    # xs[p, kc, d] = x[kc*128 + p, d]
    xs = np.ascontiguousarray(
        x.reshape(KC, P, DIM).transpose(1, 0, 2)
    ).astype(f8)

    in_maps = []
    for c in range(NCORES):
        rows = slice(c * ROWS, (c + 1) * ROWS)
        xc = x[rows]
        atq = A[rows].T.astype(f8).reshape(KC, P, ROWS)   # [kc, p, m] fp8
        im = {
            "xs": xs,
            "xtd": np.ascontiguousarray(xc.T).astype(bf16),
            "xl": np.ascontiguousarray(
                (xc - F_CONST / B_CONST).reshape(NSTRIPES, P, DIM).transpose(1, 0, 2)
            ).reshape(P, NSTRIPES * DIM).astype(bf16),
        }
        for i, (off, w, _) in enumerate(PIECES):
            im[f"at{i}"] = np.ascontiguousarray(
                atq[:, :, off : off + w].transpose(1, 0, 2)
            )
        in_maps.append(im)
    return in_maps


def run_sharded(x, A, trace=False, **kwargs):
    """Run the SPMD bass kernel; returns (full_output, BassKernelResults)."""
    from concourse.bass_utils import run_bass_kernel_spmd

    nc = _get_nc()
    res = run_bass_kernel_spmd(
        nc, _make_in_maps(x, A), core_ids=list(range(NCORES)), trace=trace, **kwargs
    )
    full = np.concatenate(
        [
            res.results[c]["out"]
            .reshape(P, NSTRIPES, DIM)
            .transpose(1, 0, 2)
            .reshape(ROWS, DIM)
            for c in range(NCORES)
        ],
        axis=0,
    )
    return full.astype(np.float32, copy=False), res


def kernel(t, x, A):
    out, _ = run_sharded(x, A)
    return out


# revision 30
# speedup vs baseline: 1.0538x; 1.0538x over previous
"""Trainium2 Bass kernel for BiochemicalDynamics.

Reference computation (f32):
    Ax    = A @ x                                   # [N, DIM]
    s     = R * rowsum(x * Ax)                      # [N, 1]
    out   = F - B*x - s                             # [N, DIM]

Strategy: compute Y^T = (A_local @ x)^T directly on the TensorEngine by
streaming A (host-side pre-transposed, fp8) as the MOVING operand
against stationary x row-chunks:

    Y^T[d, m] = sum_kc matmul(lhsT = x[kc*128:(kc+1)*128, :],     # [K, M=64]
                              rhs  = A^T[kc*128:(kc+1)*128, m])   # [K, N]

accumulated over all 64 K-chunks into a PSUM region [64, 1024].
fp8 DoubleRow packs two K-chunks per instruction (K=256), giving the PE
enough column rate to hide entirely under the A DMA stream.

The per-row dot s_i = R * <x_i, Y_i> needs a PARTITION-axis reduction
of z = x^T .* Y^T, done with tiny ones-vector matmuls (lhsT =
z[:, stripe], rhs = ones[64,1]) that land s in natural [128, .] layout.
The affine epilogue (v = F - s; out = -B*x + v) runs on the otherwise
idle VectorEngine via tensor_scalar with a per-partition scalar AP.

A is streamed m-major in four 256-row pieces: each piece's Y^T finishes
while the next piece is still streaming, so its reduction, epilogue and
output store all hide under the DMA stream — only the last piece's
short chain sits in the tail.

DMA discipline (hard-won): ALL loads go on the single SP HWDGE ring in
an order where every DMAHW sem lane's previous user completes early
(lanes are assigned round-robin across ALL HWDGE DMAs of both rings,
and a lane's next user's ISSUE is gated on its previous user's
completion). Mid-stream stores go via GPSIMD/SWDGE, which uses a
separate sem-lane pool, so their late gating can't stall the stream.

A is quantized to fp8_e4m3 host-side: its rounding error is random-sign
and averages out over the 8192-term row reductions (measured ~1.7e-3
max rel err vs the 2e-2 gate) while halving HBM traffic vs fp16 —
this kernel is DMA-bound on A (8MB/core at ~358 GB/s).

Sharding: row-shard A (1024 rows/core); every core gets the full x.
No cross-core communication.
"""

import sys

import numpy as np

for _p in ("/opt/trn_rl_repo", "/root/.axon_site/_ro/trn_rl_repo"):
    if _p not in sys.path:
        sys.path.append(_p)

N = 8192
DIM = 64
NCORES = 8
ROWS = N // NCORES       # 1024 rows of A per core
P = 128
NSTRIPES = ROWS // P     # 8 row-stripes per core
KC = N // P              # 64 contraction chunks of 128
KP = KC // 2             # 32 DoubleRow chunk-pairs

F_CONST = 1.0
B_CONST = 0.1
R_CONST = 0.01

# (m-offset, m-width, kc-chunk schedule). Widths stay within PSUM banks
# (512-aligned boundaries). The first chunk of piece 0 is small so the
# first matmul starts early; piece 3 tapers so the PE and the final
# chain catch the stream quickly at the end.
PIECES = (
    (0, 256, (8, 24, 32)),
    (256, 256, (32, 32)),
    (512, 256, (32, 32)),
    (768, 256, (32, 16, 8, 4, 4)),
)
P_STRIPES = ((0, 1), (2, 3), (4, 5), (6, 7))

_CACHE = {}


def _build_nc():
    import concourse.mybir as mybir
    import concourse.tile as tile
    from concourse import bacc

    f32 = mybir.dt.float32
    bf16 = mybir.dt.bfloat16
    f8 = mybir.dt.float8e4

    nc = bacc.Bacc(
        trn_type="TRN2", target_bir_lowering=False, debug=False, num_devices=NCORES
    )

    # at{p}[pp, kc, j] = A[rows_c[off_p + j], kc*128 + pp]  (fp8 A^T pieces)
    at_dram = [
        nc.dram_tensor(f"at{i}", [P, KC, w], f8, kind="ExternalInput")
        for i, (_, w, _) in enumerate(PIECES)
    ]
    # xs[p, kc, d] = x[kc*128 + p, d]          (stationary chunks, fp8)
    xs = nc.dram_tensor("xs", [P, KC, DIM], f8, kind="ExternalInput")
    # xtd[d, m] = x[rows_c[m], d]              (bf16, for the rowwise dot)
    xtd = nc.dram_tensor("xtd", [DIM, ROWS], bf16, kind="ExternalInput")
    # xl[p, s*64+d] = x[rows_c[s*128+p], d] - F/B   (bf16; F pre-folded so
    # the epilogue is a single tensor_scalar: out = -B*xl - s)
    xl = nc.dram_tensor("xl", [P, NSTRIPES * DIM], bf16, kind="ExternalInput")
    out = nc.dram_tensor("out", [P, NSTRIPES * DIM], f32, kind="ExternalOutput")

    mult = mybir.AluOpType.mult
    subtract = mybir.AluOpType.subtract
    DR = mybir.MatmulPerfMode.DoubleRow

    with tile.TileContext(nc) as tc:
        with (
            tc.tile_pool(name="xpool", bufs=1) as xpool,
            tc.tile_pool(name="spool", bufs=1) as spool,
            tc.tile_pool(name="psum", bufs=1, space="PSUM") as psum_pool,
        ):
            xs_sb = xpool.tile([P, KC, DIM], f8)
            xtd_sb = xpool.tile([DIM, ROWS], bf16)
            xl_sb = xpool.tile([P, NSTRIPES * DIM], bf16)
            ones_sb = xpool.tile([DIM, 1], bf16)
            nc.any.memset(ones_sb[:], 1.0)
            at_sb = [
                xpool.tile([P, KC, w], f8, name=f"at_sb{i}", tag=f"at{i}")
                for i, (_, w, _) in enumerate(PIECES)
            ]

            # Single-ring load order: the stationary piece the first
            # matmuls need, the first A chunk, the remaining small loads,
            # then the A stream.
            nc.sync.dma_start(out=xs_sb[:, 0:4, :], in_=xs[:, 0:4, :])
            w0 = PIECES[0][2][0]
            nc.sync.dma_start(out=at_sb[0][:, 0:w0, :], in_=at_dram[0][:, 0:w0, :])
            nc.sync.dma_start(out=xs_sb[:, 4:, :], in_=xs[:, 4:, :])
            nc.sync.dma_start(out=xtd_sb[:], in_=xtd[:])
            nc.sync.dma_start(out=xl_sb[:], in_=xl[:])
            for i, (_, w, chunks) in enumerate(PIECES):
                o = w0 if i == 0 else 0
                for ck in chunks[1 if i == 0 else 0 :]:
                    nc.sync.dma_start(
                        out=at_sb[i][:, o : o + ck, :],
                        in_=at_dram[i][:, o : o + ck, :],
                    )
                    o += ck

            # Y^T accumulation: [64, 1024] f32 PSUM (2 banks).
            yt_ps = psum_pool.tile([DIM, ROWS], f32, tag="yt")
            z_sb = spool.tile([DIM, ROWS], bf16, tag="z")
            s_ps = psum_pool.tile([P, NSTRIPES], f32, tag="s")
            o_sb = spool.tile([P, NSTRIPES * DIM], f32, tag="o")

            def stripe_reduce(s):
                # PE: s[p, s] = sum_d z[d, s*128 + p] via a ones-matmul.
                nc.tensor.matmul(
                    s_ps[:, s : s + 1],
                    z_sb[:, s * P : (s + 1) * P],
                    ones_sb[:],
                    start=True,
                    stop=True,
                )

            def stripe_out(s):
                # DVE: out = (xl_adj * -B) - s, with F pre-folded into
                # xl_adj = x - F/B host-side and R folded into z.
                nc.vector.tensor_scalar(
                    o_sb[:, s * DIM : (s + 1) * DIM],
                    xl_sb[:, s * DIM : (s + 1) * DIM],
                    -B_CONST,
                    s_ps[:, s : s + 1],
                    op0=mult,
                    op1=subtract,
                )

            def finish(i):
                stripes = P_STRIPES[i]
                for s in stripes:
                    stripe_reduce(s)
                if i < len(PIECES) - 1:
                    for s in stripes:
                        stripe_out(s)
                    s0, s1 = stripes[0], stripes[-1] + 1
                    nc.gpsimd.dma_start(
                        out=out[:, s0 * DIM : s1 * DIM],
                        in_=o_sb[:, s0 * DIM : s1 * DIM],
                    )
                else:
                    # Tail: store each stripe the moment its epilogue op
                    # lands, so the final (critical) store is only 32KB.
                    for s in stripes:
                        stripe_out(s)
                        nc.sync.dma_start(
                            out=out[:, s * DIM : (s + 1) * DIM],
                            in_=o_sb[:, s * DIM : (s + 1) * DIM],
                        )

            last = len(PIECES) - 1
            for i, (off, w, _) in enumerate(PIECES):
                for c in range(KP):
                    nc.tensor.matmul(
                        yt_ps[:, off : off + w],
                        xs_sb[:, 2 * c : 2 * c + 2, :],
                        at_sb[i][:, 2 * c : 2 * c + 2, :],
                        start=(c == 0),
                        stop=(c == KP - 1),
                        perf_mode=DR,
                    )
                # z[d, m] = (xtd * R) * Y^T for this piece -> bf16
                nc.vector.scalar_tensor_tensor(
                    z_sb[:, off : off + w],
                    xtd_sb[:, off : off + w],
                    R_CONST,
                    yt_ps[:, off : off + w],
                    op0=mult,
                    op1=mult,
                )
                # Finish piece i-1 AFTER piece i's matmuls so the PE
                # never stalls waiting on the DVE mid-stream.
                if i > 0:
                    finish(i - 1)
            finish(last)

    nc.finalize()
    return nc


def _get_nc():
    if "nc" not in _CACHE:
        _CACHE["nc"] = _build_nc()
    return _CACHE["nc"]


def _make_in_maps(x, A):
    import ml_dtypes

    f8 = ml_dtypes.float8_e4m3
    bf16 = ml_dtypes.bfloat16
    x = np.ascontiguousarray(np.asarray(x, dtype=np.float32))
    A = np.asarray(A, dtype=np.float32)

    # xs[p, kc, d] = x[kc*128 + p, d]
    xs = np.ascontiguousarray(
        x.reshape(KC, P, DIM).transpose(1, 0, 2)
    ).astype(f8)

    in_maps = []
    for c in range(NCORES):
        rows = slice(c * ROWS, (c + 1) * ROWS)
        xc = x[rows]
        atq = A[rows].T.astype(f8).reshape(KC, P, ROWS)   # [kc, p, m] fp8
        im = {
            "xs": xs,
            "xtd": np.ascontiguousarray(xc.T).astype(bf16),
            "xl": np.ascontiguousarray(
                (xc - F_CONST / B_CONST).reshape(NSTRIPES, P, DIM).transpose(1, 0, 2)
            ).reshape(P, NSTRIPES * DIM).astype(bf16),
        }
        for i, (off, w, _) in enumerate(PIECES):
            im[f"at{i}"] = np.ascontiguousarray(
                atq[:, :, off : off + w].transpose(1, 0, 2)
            )
        in_maps.append(im)
    return in_maps


def run_sharded(x, A, trace=False, **kwargs):
    """Run the SPMD bass kernel; returns (full_output, BassKernelResults)."""
    from concourse.bass_utils import run_bass_kernel_spmd

    nc = _get_nc()
    res = run_bass_kernel_spmd(
        nc, _make_in_maps(x, A), core_ids=list(range(NCORES)), trace=trace, **kwargs
    )
    full = np.concatenate(
        [
            res.results[c]["out"]
            .reshape(P, NSTRIPES, DIM)
            .transpose(1, 0, 2)
            .reshape(ROWS, DIM)
            for c in range(NCORES)
        ],
        axis=0,
    )
    return full.astype(np.float32, copy=False), res


def kernel(t, x, A):
    out, _ = run_sharded(x, A)
    return out
